# revision 1
# baseline (speedup 1.0000x reference)
"""MoE FFN (FMoE) kernel for 8 Trainium2 NeuronCores.

Problem: N=4096 tokens, D=512, H=2048, E=8 experts, top_k=2.
  logits = inp @ gate_w + gate_b ; top-2 softmax -> combine weights
  out = sum_e combine[:, e] * (gelu_tanh(inp @ w1[e] + b1[e]) @ w2[e] + b2[e])

Strategy (expert parallelism, `build_moe`): core e owns expert e's
weights (bf16). The gate runs data-parallel in exact fp32 (each core
gates its own 512 tokens; the tightest 2nd-vs-3rd logit margin in this
data is 6e-8, so top-2 selection must match the reference's fp32
bit-for-bit — the PE fp32 matmul does). Top-2 (idx0, idx1, w0, w1) per
token is AllGathered (8KB/core), from which every core derives its own
expert's mask + combine weight for all N tokens. Tokens are compacted
per half (2048 tokens -> <=640 slots) via matmul prefix-sum + ONE
multi-column indirect meta scatter, then a fused dma_gather(transpose)
pulls the selected x rows from DRAM directly into the transposed
[128, DC, 640] bf16 layout layer 1 wants. The 2-layer gelu FFN runs in
bf16 (PE full rate), layer-2 output is gate-scaled and dma_scatter_add
-ed into a zero-filled bf16 [2048, D] per-half partial; a
ReduceScatter(add) per half (the second overlapping the other half's
FFN) leaves each core with 2x256 output rows, reassembled on host.

`build_dense` (unused fallback) is the routing-free data-parallel
variant: every core computes all 8 experts for its 512 tokens.
"""
import numpy as np

import concourse.bacc as bacc
import concourse.bass as bass
import concourse.mybir as mybir
import concourse.tile as tile
from concourse.bass_utils import run_bass_kernel_spmd
from concourse.masks import make_identity

N, D, H, E, TOPK = 4096, 512, 2048, 8, 2
M = 8              # cores
TN = N // M        # tokens per core
P = 128
DC = D // P        # 4 contraction chunks over D
HC = H // P        # 16 chunks over H
TC = TN // P       # 4 token tiles per core
NT = N // P        # 32 token tiles total

NH = N // 2        # tokens per half (2048)
HT = NT // 2       # 16 token tiles per half
CAPH = 640         # compact slots per half (max observed load 559)
SCH = CAPH // P    # 5 compact tiles per half
CCS = [(0, 384), (384, 640)]   # layer-1 moving-dim chunks (PSUM bank <=512 fp32)
BIG = 8192.0       # OOB sentinel for unselected tokens

FP32 = mybir.dt.float32
BF16 = mybir.dt.bfloat16
I16 = mybir.dt.int16
I32 = mybir.dt.int32

AFT = mybir.ActivationFunctionType


DEBUG = False


def build_moe():
    nc = bacc.Bacc(None, target_bir_lowering=False)

    xT_own = nc.dram_tensor("xT_own", [D, N], FP32, kind="ExternalInput")
    x_bf = nc.dram_tensor("x_bf", [N, D], BF16, kind="ExternalInput")
    gate_w = nc.dram_tensor("gate_w", [D, E], FP32, kind="ExternalInput")
    gate_b = nc.dram_tensor("gate_b", [1, E], FP32, kind="ExternalInput")
    w1h_in = nc.dram_tensor("w1h_in", [P, HC, DC, P], BF16, kind="ExternalInput")
    b1t_in = nc.dram_tensor("b1t_in", [P, HC], FP32, kind="ExternalInput")
    w2e = nc.dram_tensor("w2e", [H, D], BF16, kind="ExternalInput")
    b2r_in = nc.dram_tensor("b2r_in", [1, D], BF16, kind="ExternalInput")
    ones_in = nc.dram_tensor("ones_in", [1, P], BF16, kind="ExternalInput")
    triu_in = nc.dram_tensor("triu_in", [P, P], FP32, kind="ExternalInput")
    tokid_in = nc.dram_tensor("tokid_in", [P, NT], FP32, kind="ExternalInput")
    dumpc_in = nc.dram_tensor("dumpc_in", [P, NT], FP32, kind="ExternalInput")
    dump16_in = nc.dram_tensor("dump16_in", [16, CAPH // 16], FP32,
                               kind="ExternalInput")
    b16_in = nc.dram_tensor("b16_in", [16, P], FP32, kind="ExternalInput")
    eid_in = nc.dram_tensor("eid_in", [P, 1], FP32, kind="ExternalInput")

    # compact meta: rows [0, CAPH) = slots, rows [CAPH, CAPH+NH) = dump for
    # unselected tokens. Lane 0 = tokid, lane 1 = gate weight (256B rows for
    # dma_scatter_add's elem-size floor).
    cmetas = [nc.dram_tensor(f"cmeta{h}", [CAPH + NH, 64], FP32)
              for h in range(2)]
    offds = [nc.dram_tensor(f"offd{h}", [NH], FP32) for h in range(2)]
    # rows [NH, NH+P) are a dump area for pad-slot writes: concurrent CCE adds
    # to one row are read-modify-write and can drop a racing real add, so pads
    # must never share a row with real tokens.
    partials = [nc.dram_tensor(f"partial{h}", [NH + P, D], BF16)
                for h in range(2)]
    rss = [nc.dram_tensor(f"rs{h}", [NH // M, D], BF16) for h in range(2)]
    outs = [nc.dram_tensor(f"o{h}", [NH // M, D], BF16, kind="ExternalOutput")
            for h in range(2)]
    if DEBUG:
        d_msb = nc.dram_tensor("d_msb", [P, 2, SCH, 2], FP32, kind="ExternalOutput")
        d_idx = nc.dram_tensor("d_idx", [P, 2, CAPH // 16], I16,
                               kind="ExternalOutput")
        d_xtg = nc.dram_tensor("d_xtg", [P, 2, DC, CAPH], BF16,
                               kind="ExternalOutput")
        d_y = nc.dram_tensor("d_y", [P, 2, SCH, D], BF16, kind="ExternalOutput")
        d_part = nc.dram_tensor("d_part", [P, 2, D], BF16, kind="ExternalOutput")

    with tile.TileContext(nc) as tc:
        with (
            tc.tile_pool(name="const", bufs=1) as const,
            tc.tile_pool(name="xsp", bufs=DC) as xsp,
            tc.tile_pool(name="gatep", bufs=2) as gatep,
            tc.tile_pool(name="routep", bufs=1) as routep,
            tc.tile_pool(name="w1p", bufs=HC) as w1p,
            tc.tile_pool(name="w2p", bufs=HC) as w2p,
            tc.tile_pool(name="xtgp", bufs=2) as xtgp,
            tc.tile_pool(name="hp", bufs=2 * HC) as hp,
            tc.tile_pool(name="yp", bufs=2) as yp,
            tc.tile_pool(name="psG", bufs=2, space="PSUM") as psG,
            tc.tile_pool(name="ps1", bufs=3, space="PSUM") as ps1,
            tc.tile_pool(name="ps2", bufs=3, space="PSUM") as ps2,
        ):
            # ---- gate input first: it heads the critical path ----
            gws = []
            for dc in range(DC):
                g = const.tile([P, E], FP32, tag=f"gw{dc}")
                nc.sync.dma_start(g[:], gate_w[dc * P:(dc + 1) * P, :])
                gws.append(g)
            gb = const.tile([1, E], FP32)
            nc.sync.dma_start(gb[:], gate_b[:])

            # ---- constants ----
            ones_row = const.tile([1, TN], FP32)
            nc.vector.memset(ones_row[:], 1.0)
            ones_col = const.tile([P, 1], FP32)
            nc.vector.memset(ones_col[:], 1.0)
            ones_s = const.tile([1, P], FP32)
            nc.vector.memset(ones_s[:], 1.0)
            ones_r = const.tile([1, P], BF16)
            nc.sync.dma_start(ones_r[:], ones_in[:])
            ident = const.tile([P, P], FP32)
            make_identity(nc, ident[:])
            triu = const.tile([P, P], FP32)
            nc.sync.dma_start(triu[:], triu_in[:])
            tokid = const.tile([P, NT], FP32)
            nc.sync.dma_start(tokid[:], tokid_in[:])
            dumpc = const.tile([P, NT], FP32)
            nc.sync.dma_start(dumpc[:], dumpc_in[:])
            dump16 = const.tile([16, CAPH // 16], FP32)
            nc.sync.dma_start(dump16[:], dump16_in[:])
            b16 = const.tile([16, P], FP32)
            nc.sync.dma_start(b16[:], b16_in[:])
            eidf = const.tile([P, 1], FP32)
            nc.sync.dma_start(eidf[:], eid_in[:])
            eidu = const.tile([P, 1], mybir.dt.uint32)
            nc.vector.tensor_copy(eidu[:], eidf[:])
            b1t = const.tile([P, HC], FP32)
            nc.sync.dma_start(b1t[:], b1t_in[:])
            b2r = const.tile([1, D], BF16)
            nc.sync.dma_start(b2r[:], b2r_in[:])

            # zero-init meta slot rows + output partials (off critical path)
            zmeta = const.tile([P, SCH, 64], FP32)
            nc.vector.memset(zmeta[:], 0.0)
            for h in range(2):
                nc.sync.dma_start(
                    cmetas[h][0:CAPH].rearrange("(s p) c -> p s c", p=P),
                    zmeta[:])
            ztb = const.tile([P, D], BF16)
            nc.vector.memset(ztb[:], 0.0)
            for h in range(2):
                for j in range(NH // P):
                    nc.sync.dma_start(partials[h][j * P:(j + 1) * P, :], ztb[:])

            # resident expert weights (bf16)
            w2t = []
            for hh in range(HC):
                w = w2p.tile([P, D], BF16, tag="w2t")
                nc.sync.dma_start(w[:], w2e[hh * P:(hh + 1) * P, :])
                w2t.append(w)
            w1t = []
            for hh in range(HC):
                w = w1p.tile([P, DC, P], BF16, tag="w1t")
                nc.sync.dma_start(w[:], w1h_in[:, hh])
                w1t.append(w)

            # ---- replicated gate: all N tokens, exact fp32, 512-tok chunks ----
            m_pack = routep.tile([P, NT], FP32, tag="m_pack")
            wt_pack = routep.tile([P, NT], FP32, tag="wt_pack")
            for ch in range(N // TN):
                xts = []
                for dc in range(DC):
                    t_ = xsp.tile([P, TN], FP32, tag="xts")
                    nc.sync.dma_start(
                        t_[:],
                        xT_own[dc * P:(dc + 1) * P, ch * TN:(ch + 1) * TN])
                    xts.append(t_)
                psT = psG.tile([E, TN], FP32, tag="psG")
                for dc in range(DC):
                    nc.tensor.matmul(psT[:], gws[dc][:], xts[dc][:],
                                     start=(dc == 0), stop=False)
                nc.tensor.matmul(psT[:], gb[:], ones_row[:],
                                 start=False, stop=True)
                lgT = gatep.tile([E, TN], FP32, tag="lgT")
                nc.vector.tensor_copy(lgT[:], psT[:])

                mxp = gatep.tile([P, TC, 8], FP32, tag="mxp")
                ixp = gatep.tile([P, TC, 8], mybir.dt.uint32, tag="ixp")
                for k in range(TC):
                    plg = psG.tile([P, E], FP32, tag="psG")
                    nc.tensor.transpose(plg[:], lgT[:, k * P:(k + 1) * P],
                                        ident[:E, :E])
                    lg = gatep.tile([P, E], FP32, tag="lg")
                    nc.vector.tensor_copy(lg[:], plg[:])
                    nc.vector.max_with_indices(mxp[:, k, :], ixp[:, k, :], lg[:])

                csl = slice(ch * TC, (ch + 1) * TC)
                dlt = gatep.tile([P, TC], FP32, tag="dlt")
                nc.vector.tensor_sub(dlt[:], mxp[:, :, 1], mxp[:, :, 0])
                e1 = gatep.tile([P, TC], FP32, tag="e1")
                nc.scalar.activation(e1[:], dlt[:], AFT.Exp)
                den = gatep.tile([P, TC], FP32, tag="den")
                nc.vector.tensor_scalar_add(den[:], e1[:], 1.0)
                w0 = gatep.tile([P, TC], FP32, tag="w0")
                nc.vector.reciprocal(w0[:], den[:])
                w1_ = gatep.tile([P, TC], FP32, tag="w1_")
                nc.vector.tensor_mul(w1_[:], e1[:], w0[:])
                h0 = gatep.tile([P, TC], FP32, tag="h0")
                nc.vector.tensor_tensor(
                    out=h0[:], in0=ixp[:, :, 0],
                    in1=eidu[:].to_broadcast([P, TC]),
                    op=mybir.AluOpType.is_equal)
                h1 = gatep.tile([P, TC], FP32, tag="h1")
                nc.vector.tensor_tensor(
                    out=h1[:], in0=ixp[:, :, 1],
                    in1=eidu[:].to_broadcast([P, TC]),
                    op=mybir.AluOpType.is_equal)
                nc.vector.tensor_add(m_pack[:, csl], h0[:], h1[:])
                nc.vector.tensor_mul(h0[:], h0[:], w0[:])
                nc.vector.tensor_mul(h1[:], h1[:], w1_[:])
                nc.vector.tensor_add(wt_pack[:, csl], h0[:], h1[:])

            # ---- routing per half ----
            # prefix-sum -> per-token slot (unselected -> dump region) ->
            # 16-wrap idx via DRAM bounce + PE replicate -> ONE meta
            # dma_scatter_add -> slot->tokid idx -> fused gather+transpose.
            xtgs, msbs, idxs, idxs_s = [], [], [], []
            for half in range(2):
                hsl = slice(HT * half, HT * (half + 1))
                p_tot = psG.tile([HT, 1], FP32, tag="psG")
                nc.tensor.matmul(p_tot[:], m_pack[:, hsl], ones_col[:],
                                 start=True, stop=True)
                totT = routep.tile([HT, 1], FP32, tag=f"totT{half}")
                nc.vector.tensor_copy(totT[:], p_tot[:])
                p_srow = psG.tile([1, HT], FP32, tag="psG")
                nc.tensor.matmul(p_srow[:], totT[:], triu[0:HT, 0:HT],
                                 start=True, stop=True)
                s_row = routep.tile([1, HT], FP32, tag=f"srow{half}")
                nc.vector.tensor_copy(s_row[:], p_srow[:])
                p_pl = psG.tile([P, HT], FP32, tag="psG")
                nc.tensor.matmul(p_pl[:], triu[:], m_pack[:, hsl],
                                 start=True, stop=False)
                nc.tensor.matmul(p_pl[:], ones_s[:], s_row[:], start=False, stop=True)

                # off = m*slot + (1-m)*(CAPH + tokid)  (per token, fp32)
                off_f = routep.tile([P, HT], FP32, tag=f"offf{half}")
                nc.vector.tensor_sub(off_f[:], p_pl[:], dumpc[:, hsl])
                nc.vector.tensor_mul(off_f[:], off_f[:], m_pack[:, hsl])
                nc.vector.tensor_add(off_f[:], off_f[:], dumpc[:, hsl])
                # DRAM bounce into token order, reload 16-wrapped
                nc.sync.dma_start(
                    offds[half].rearrange("(t p) -> p t", p=P), off_f[:])
                offw = routep.tile([16, P], FP32, tag=f"offw{half}")
                nc.sync.dma_start(
                    offw[:], offds[half].rearrange("(m q) -> q m", q=16))
                ps_sx = psG.tile([P, P], FP32, tag="psG")
                nc.tensor.matmul(ps_sx[:], b16[:], offw[:], start=True, stop=True)
                idx_sx = routep.tile([P, P], I16, tag=f"idxsx{half}")
                nc.vector.tensor_copy(idx_sx[:], ps_sx[:])

                # meta payload: lane0 = tokid, lane1 = gate weight
                vals64 = routep.tile([P, HT, 64], FP32, tag=f"vals{half}")
                nc.vector.memset(vals64[:], 0.0)
                nc.vector.tensor_copy(vals64[:, :, 0], tokid[:, hsl])
                nc.vector.tensor_copy(vals64[:, :, 1], wt_pack[:, hsl])
                nc.gpsimd.dma_scatter_add(
                    cmetas[half][:], vals64[:], idx_sx[:], NH, NH, 64)

                # meta back: weights in 128-wrap, tokids in 16-wrap
                msb = routep.tile([P, SCH, 64], FP32, tag=f"msb{half}")
                nc.sync.dma_start(
                    msb[:], cmetas[half][0:CAPH].rearrange("(s p) c -> p s c", p=P))
                msbs.append(msb)
                m16 = routep.tile([16, CAPH // 16, 64], FP32, tag=f"m16_{half}")
                nc.sync.dma_start(
                    m16[:], cmetas[half][0:CAPH].rearrange("(s p) c -> p s c", p=16))
                mt = routep.tile([16, CAPH // 16], FP32, tag=f"mt{half}")
                nc.vector.tensor_copy(mt[:], m16[:, :, 0])
                ps_g = psG.tile([P, CAPH // 16], FP32, tag="psG")
                nc.tensor.matmul(ps_g[:], b16[:], mt[:], start=True, stop=True)
                idx_g = routep.tile([P, CAPH // 16], I16, tag=f"idxg{half}")
                nc.vector.tensor_copy(idx_g[:], ps_g[:])
                idxs.append(idx_g)
                # scatter idx: pads (wt==0) diverted to the dump rows
                pad16 = routep.tile([16, CAPH // 16], FP32, tag=f"pad16_{half}")
                nc.vector.tensor_scalar(pad16[:], m16[:, :, 1], 0.0, None,
                                        op0=mybir.AluOpType.is_equal)
                nc.vector.tensor_mul(pad16[:], pad16[:], dump16[:])
                mts = routep.tile([16, CAPH // 16], FP32, tag=f"mts{half}")
                nc.vector.tensor_add(mts[:], mt[:], pad16[:])
                ps_s = psG.tile([P, CAPH // 16], FP32, tag="psG")
                nc.tensor.matmul(ps_s[:], b16[:], mts[:], start=True, stop=True)
                idx_s = routep.tile([P, CAPH // 16], I16, tag=f"idxs{half}")
                nc.vector.tensor_copy(idx_s[:], ps_s[:])
                idxs_s.append(idx_s)

                xtg = xtgp.tile([P, DC, CAPH], BF16, tag="xtg")
                nc.gpsimd.dma_gather(
                    xtg[:], x_bf[NH * half:NH * (half + 1), :], idx_g[:],
                    CAPH, CAPH, D, transpose=True)
                xtgs.append(xtg)
                if DEBUG:
                    nc.sync.dma_start(d_msb[:, half], msb[:, :, 0:2])
                    nc.sync.dma_start(d_idx[:, half], idx_g[:])
                    nc.sync.dma_start(d_xtg[:, half], xtg[:])

            # ---- FFN per half (bf16), scatter-add, ReduceScatter ----
            for half in range(2):
                xtg, msb, idx16 = xtgs[half], msbs[half], idxs_s[half]
                hts = []
                for hh in range(HC):
                    ht = hp.tile([P, CAPH], BF16, tag="ht")
                    pcs = [ps1.tile([P, c1 - c0], FP32, tag="ps1", name=f"pcs{ci}")
                           for ci, (c0, c1) in enumerate(CCS)]
                    for dc in range(DC):
                        for ci, (c0, c1) in enumerate(CCS):
                            nc.tensor.matmul(
                                pcs[ci][:], w1t[hh][:, dc, :], xtg[:, dc, c0:c1],
                                start=(dc == 0), stop=(dc == DC - 1))
                    for ci, (c0, c1) in enumerate(CCS):
                        nc.scalar.activation(ht[:, c0:c1], pcs[ci][:],
                                             AFT.Gelu_apprx_tanh,
                                             bias=b1t[:, hh:hh + 1])
                    hts.append(ht)

                y = yp.tile([P, SCH, D], BF16, tag="y")
                for s in range(SCH):
                    p2 = ps2.tile([P, D], FP32, tag="ps2")
                    for hh in range(HC):
                        nc.tensor.matmul(p2[:], hts[hh][:, s * P:(s + 1) * P],
                                         w2t[hh][:], start=(hh == 0), stop=False)
                    nc.tensor.matmul(p2[:], ones_r[:], b2r[:],
                                     start=False, stop=True)
                    nc.scalar.activation(y[:, s, :], p2[:], AFT.Copy,
                                         scale=msb[:, s, 1:2])

                if DEBUG:
                    nc.sync.dma_start(d_y[:, half], y[:])
                nc.gpsimd.dma_scatter_add(
                    partials[half][:], y[:], idx16[:], CAPH, CAPH, D)
                if DEBUG:
                    pb = yp.tile([P, D], BF16, tag="pb")
                    nc.sync.dma_start(pb[:], partials[half][0:P, :])
                    nc.sync.dma_start(d_part[:, half], pb[:])
                nc.gpsimd.collective_compute(
                    "ReduceScatter", mybir.AluOpType.add,
                    replica_groups=[list(range(M))],
                    ins=[partials[half][0:NH].opt()], outs=[rss[half][:].opt()])
                for j in range(NH // M // P):
                    ob = yp.tile([P, D], BF16, tag="ob")
                    nc.sync.dma_start(ob[:], rss[half][j * P:(j + 1) * P, :])
                    nc.sync.dma_start(outs[half][j * P:(j + 1) * P, :], ob[:])

    nc.compile()
    return nc


def make_moe_in_maps(inp, gate_w, gate_b, w1, b1, w2, b2):
    import ml_dtypes
    bf16 = ml_dtypes.bfloat16
    inp = np.ascontiguousarray(np.asarray(inp, dtype=np.float32))
    gate_w = np.ascontiguousarray(np.asarray(gate_w, dtype=np.float32))
    gate_b = np.ascontiguousarray(np.asarray(gate_b, dtype=np.float32)).reshape(1, E)
    w1 = np.asarray(w1, dtype=np.float32)
    b1 = np.asarray(b1, dtype=np.float32)
    w2 = np.asarray(w2, dtype=np.float32)
    b2 = np.asarray(b2, dtype=np.float32)

    x_bf = np.ascontiguousarray(inp.astype(bf16))
    xT = np.ascontiguousarray(inp.T)
    triu = np.triu(np.ones((P, P), np.float32), k=1)
    # token id within its half: tile t holds tokens (t%16)*128+p of half t//16
    tokid = ((np.arange(NT)[None, :] % HT) * P
             + np.arange(P)[:, None]).astype(np.float32)
    dumpc = tokid + CAPH
    # pad-slot scatter target: NH + slot%P, distinct rows past the RS window
    slot16 = (np.arange(CAPH // 16)[None, :] * 16 + np.arange(16)[:, None])
    dump16 = (NH + slot16 % P).astype(np.float32)
    # replication matrix: b16[k, i] = 1 iff i % 16 == k (16->128 partition bcast)
    b16 = (np.arange(P)[None, :] % 16 == np.arange(16)[:, None]).astype(np.float32)
    ones = np.ones((1, P), np.float32).astype(bf16)

    in_maps = []
    for c in range(M):
        w1h = np.ascontiguousarray(
            w1[c].reshape(DC, P, HC, P).transpose(1, 2, 0, 3).astype(bf16))
        in_maps.append({
            "xT_own": xT,
            "x_bf": x_bf,
            "gate_w": gate_w, "gate_b": gate_b,
            "w1h_in": w1h,
            "b1t_in": np.ascontiguousarray(b1[c].reshape(HC, P).T),
            "w2e": np.ascontiguousarray(w2[c].astype(bf16)),
            "b2r_in": np.ascontiguousarray(b2[c].reshape(1, D).astype(bf16)),
            "ones_in": ones,
            "triu_in": triu,
            "tokid_in": tokid,
            "dumpc_in": dumpc,
            "dump16_in": dump16,
            "b16_in": b16,
            "eid_in": np.full((P, 1), c, np.float32),
        })
    return in_maps


# ---------------------------------------------------------------------------
# Fallback: dense data-parallel variant (every core runs all 8 experts on its
# 512 tokens). Unused unless KERNEL_KIND is changed.
# ---------------------------------------------------------------------------

def _gate_combine(nc, tc_ctx, pools, xts, gws, gb, ones_s, iota_u, n_tok_chunks):
    gatep, cmbp, psg = pools
    U32 = mybir.dt.uint32
    TNW = n_tok_chunks * P
    ones_row = gatep.tile([1, TNW], FP32, tag="ones_row")
    nc.vector.memset(ones_row[:], 1.0)
    ident = gatep.tile([P, P], FP32, tag="ident_g")
    make_identity(nc, ident[:])
    psT = psg.tile([E, TNW], FP32, tag="psg")
    for dc in range(len(xts)):
        nc.tensor.matmul(psT[:], gws[dc][:], xts[dc][:, 0:TNW],
                         start=(dc == 0), stop=False)
    nc.tensor.matmul(psT[:], gb[:], ones_row[:], start=False, stop=True)
    lgT = gatep.tile([E, TNW], FP32, tag="lgT")
    nc.scalar.activation(lgT[:], psT[:], AFT.Copy)

    cmb = []
    cmbT = []
    for t in range(n_tok_chunks):
        pg = psg.tile([P, E], FP32, tag="psg")
        nc.tensor.transpose(pg[:], lgT[:, t * P:(t + 1) * P], ident[:E, :E])

        lg = gatep.tile([P, E], FP32, tag="lg")
        nc.vector.tensor_copy(lg[:], pg[:])
        mx = gatep.tile([P, 8], FP32, tag="mx")
        ix = gatep.tile([P, 8], U32, tag="ix")
        nc.vector.max_with_indices(mx[:], ix[:], lg[:])

        dlt = gatep.tile([P, 1], FP32, tag="dlt")
        nc.vector.tensor_sub(dlt[:], mx[:, 1:2], mx[:, 0:1])
        e1 = gatep.tile([P, 1], FP32, tag="e1")
        nc.scalar.activation(e1[:], dlt[:], AFT.Exp)
        den = gatep.tile([P, 1], FP32, tag="den")
        nc.vector.tensor_scalar_add(den[:], e1[:], 1.0)
        w0 = gatep.tile([P, 1], FP32, tag="w0")
        nc.vector.reciprocal(w0[:], den[:])
        w1_ = gatep.tile([P, 1], FP32, tag="w1_")
        nc.vector.tensor_mul(w1_[:], e1[:], w0[:])

        oh0 = gatep.tile([P, E], FP32, tag="oh0")
        nc.vector.tensor_tensor(out=oh0[:], in0=ix[:, 0:1].to_broadcast([P, E]),
                                in1=iota_u[:], op=mybir.AluOpType.is_equal)
        oh1 = gatep.tile([P, E], FP32, tag="oh1")
        nc.vector.tensor_tensor(out=oh1[:], in0=ix[:, 1:2].to_broadcast([P, E]),
                                in1=iota_u[:], op=mybir.AluOpType.is_equal)
        nc.vector.tensor_scalar_mul(oh0[:], oh0[:], w0[:, 0:1])
        nc.vector.tensor_scalar_mul(oh1[:], oh1[:], w1_[:, 0:1])
        c = cmbp.tile([P, E], FP32, tag="cmb")
        nc.vector.tensor_add(c[:], oh0[:], oh1[:])
        cmb.append(c)
        pct = psg.tile([E, P], FP32, tag="psg")
        nc.tensor.transpose(pct[:], c[:], ident[:])
        ct = cmbp.tile([E, P], BF16, tag="cmbT")
        nc.vector.tensor_copy(ct[:], pct[:])
        cmbT.append(ct)
    return cmb, cmbT


def build_dense():
    nc = bacc.Bacc(None, target_bir_lowering=False)
    U32 = mybir.dt.uint32

    xT_r = nc.dram_tensor("xT_r", [D, TN], BF16, kind="ExternalInput")
    xT_s = nc.dram_tensor("xT_s", [D, TN], FP32, kind="ExternalInput")
    gate_w = nc.dram_tensor("gate_w", [D, E], FP32, kind="ExternalInput")
    gate_b = nc.dram_tensor("gate_b", [1, E], FP32, kind="ExternalInput")
    w1 = nc.dram_tensor("w1", [E, D, H], BF16, kind="ExternalInput")
    b1p = nc.dram_tensor("b1p", [E, P, HC], FP32, kind="ExternalInput")
    w2 = nc.dram_tensor("w2", [E, H, D], BF16, kind="ExternalInput")
    b2 = nc.dram_tensor("b2", [E, 1, D], BF16, kind="ExternalInput")
    ones_in = nc.dram_tensor("ones_in", [1, P], BF16, kind="ExternalInput")
    out = nc.dram_tensor("out", [TN, D], FP32, kind="ExternalOutput")

    with tile.TileContext(nc) as tc:
        with (
            tc.tile_pool(name="xpool", bufs=DC) as xpool,
            tc.tile_pool(name="const", bufs=1) as const,
            tc.tile_pool(name="gatep", bufs=2) as gatep,
            tc.tile_pool(name="cmbp", bufs=TC) as cmbp,
            tc.tile_pool(name="w1p", bufs=6) as w1p,
            tc.tile_pool(name="w2p", bufs=2 * HC) as w2p,
            tc.tile_pool(name="hp", bufs=2 * HC) as hp,
            tc.tile_pool(name="accp", bufs=TC) as accp,
            tc.tile_pool(name="tmpp", bufs=3) as tmpp,
            tc.tile_pool(name="bp", bufs=4) as bp,
            tc.tile_pool(name="psg", bufs=1, space="PSUM") as psg,
            tc.tile_pool(name="ps1", bufs=3, space="PSUM") as ps1,
            tc.tile_pool(name="ps2", bufs=3, space="PSUM") as ps2,
        ):
            xtr, xts = [], []
            for dc in range(DC):
                tr = xpool.tile([P, TN], BF16, tag="xtr")
                nc.sync.dma_start(tr[:], xT_r[dc * P:(dc + 1) * P, :])
                xtr.append(tr)
                ts = xpool.tile([P, TN], FP32, tag="xts")
                nc.sync.dma_start(ts[:], xT_s[dc * P:(dc + 1) * P, :])
                xts.append(ts)

            ones_s = const.tile([1, P], FP32)
            nc.vector.memset(ones_s[:], 1.0)
            ones_r = const.tile([1, P], BF16)
            nc.sync.dma_start(ones_r[:], ones_in[:])
            iota_u = const.tile([P, E], U32)
            nc.gpsimd.iota(iota_u[:], pattern=[[1, E]], base=0, channel_multiplier=0)

            gws = []
            for dc in range(DC):
                g = const.tile([P, E], FP32, tag=f"gw{dc}")
                nc.sync.dma_start(g[:], gate_w[dc * P:(dc + 1) * P, :])
                gws.append(g)
            gb = const.tile([1, E], FP32)
            nc.sync.dma_start(gb[:], gate_b[:])

            cmb, cmbT = _gate_combine(nc, tc, (gatep, cmbp, psg), xts, gws, gb,
                                      ones_s, iota_u, TC)
            b2all = bp.tile([E, D], BF16, tag="b2all")
            nc.sync.dma_start(b2all[:], b2[:, 0, :])

            acc = [None] * TC
            for e in range(E):
                w2t = []
                for h in range(HC):
                    w = w2p.tile([P, D], BF16, tag="w2t")
                    nc.sync.dma_start(w[:], w2[e, h * P:(h + 1) * P, :])
                    w2t.append(w)
                b1te = bp.tile([P, HC], FP32, tag="b1t")
                nc.sync.dma_start(b1te[:], b1p[e])

                hts = []
                w1e = w1[e].rearrange("(dc p) h -> p dc h", p=P)
                for h in range(HC):
                    w1te = w1p.tile([P, DC, P], BF16, tag="w1t")
                    nc.sync.dma_start(w1te[:], w1e[:, :, h * P:(h + 1) * P])
                    p1 = ps1.tile([P, TN], FP32)
                    for dc in range(DC):
                        nc.tensor.matmul(p1[:], w1te[:, dc, :], xtr[dc][:],
                                         start=(dc == 0), stop=(dc == DC - 1))
                    ht = hp.tile([P, TN], BF16, tag="ht")
                    nc.scalar.activation(ht[:], p1[:], AFT.Gelu_apprx_tanh,
                                         bias=b1te[:, h:h + 1])
                    hts.append(ht)

                for t in range(TC):
                    p2 = ps2.tile([P, D], FP32)
                    for h in range(HC):
                        nc.tensor.matmul(p2[:], hts[h][:, t * P:(t + 1) * P], w2t[h][:],
                                         start=(h == 0), stop=(h == HC - 1))
                    if e == 0:
                        a = accp.tile([P, D], FP32, tag="acc")
                        nc.vector.tensor_scalar_mul(a[:], p2[:], cmb[t][:, e:e + 1])
                        acc[t] = a
                    else:
                        tmp = tmpp.tile([P, D], FP32, tag="tmp")
                        nc.scalar.activation(tmp[:], p2[:], AFT.Copy,
                                             scale=cmb[t][:, e:e + 1])
                        nc.vector.tensor_add(acc[t][:], acc[t][:], tmp[:])

            for t in range(TC):
                pB = ps2.tile([P, D], FP32, tag="p2")
                nc.tensor.matmul(pB[:], cmbT[t][:], b2all[:], start=True, stop=True)
                nc.vector.tensor_add(acc[t][:], acc[t][:], pB[:])
                nc.sync.dma_start(out[t * P:(t + 1) * P, :], acc[t][:])

    nc.compile()
    return nc


def make_in_maps(inp, gate_w, gate_b, w1, b1, w2, b2):
    import ml_dtypes
    bf16 = ml_dtypes.bfloat16
    inp = np.ascontiguousarray(np.asarray(inp, dtype=np.float32))
    gate_w = np.ascontiguousarray(np.asarray(gate_w, dtype=np.float32))
    gate_b = np.ascontiguousarray(np.asarray(gate_b, dtype=np.float32)).reshape(1, E)
    w1 = np.ascontiguousarray(np.asarray(w1, dtype=np.float32).astype(bf16))
    b1 = np.asarray(b1, dtype=np.float32)
    w2 = np.ascontiguousarray(np.asarray(w2, dtype=np.float32).astype(bf16))
    b2 = np.ascontiguousarray(
        np.asarray(b2, dtype=np.float32).astype(bf16)).reshape(E, 1, D)
    b1p = np.ascontiguousarray(b1.reshape(E, HC, P).transpose(0, 2, 1))

    in_maps = []
    for c in range(M):
        xT = np.ascontiguousarray(inp[c * TN:(c + 1) * TN, :].T)
        in_maps.append({
            "xT_r": np.ascontiguousarray(xT.astype(bf16)), "xT_s": xT,
            "gate_w": gate_w, "gate_b": gate_b,
            "w1": w1, "b1p": b1p, "w2": w2, "b2": b2,
            "ones_in": np.ones((1, P), np.float32).astype(bf16),
        })
    return in_maps


_NC_CACHE = {}

# "dense" (286us) still beats the expert-parallel "moe" path (325-358us):
# the moe FFN itself is ~4x cheaper, but collective setup (~15-30us each),
# serial gpsimd scatter/gather desc-gen, and routing latency dominate.
KERNEL_KIND = "dense"


def _get_nc():
    if KERNEL_KIND not in _NC_CACHE:
        _NC_CACHE[KERNEL_KIND] = (
            build_moe() if KERNEL_KIND == "moe" else build_dense())
    return _NC_CACHE[KERNEL_KIND]


def run(inputs, trace=False, **spmd_kwargs):
    nc = _get_nc()
    mk = make_moe_in_maps if KERNEL_KIND == "moe" else make_in_maps
    in_maps = mk(
        inputs["inp"], inputs["gate_w"], inputs["gate_b"],
        inputs["w1"], inputs["b1"], inputs["w2"], inputs["b2"])
    res = run_bass_kernel_spmd(nc, in_maps, list(range(M)), trace=trace,
                               **spmd_kwargs)
    if KERNEL_KIND == "moe":
        h0 = np.concatenate(
            [np.asarray(res.results[c]["o0"], np.float32) for c in range(M)], axis=0)
        h1 = np.concatenate(
            [np.asarray(res.results[c]["o1"], np.float32) for c in range(M)], axis=0)
        out = np.concatenate([h0, h1], axis=0)
    else:
        out = np.concatenate([res.results[c]["out"] for c in range(M)], axis=0)
    return out, res


def kernel(inp, gate_w, gate_b, w1, b1, w2, b2, top_k):
    assert int(top_k) == TOPK
    out, _ = run({"inp": inp, "gate_w": gate_w, "gate_b": gate_b,
                  "w1": w1, "b1": b1, "w2": w2, "b2": b2})
    return out



# revision 2
# speedup vs baseline: 1.5685x; 1.5685x over previous
"""MoE FFN (FMoE) kernel for 8 Trainium2 NeuronCores.

Problem: N=4096 tokens, D=512, H=2048, E=8 experts, top_k=2.
  logits = inp @ gate_w + gate_b ; top-2 softmax -> combine weights
  out = sum_e combine[:, e] * (gelu_tanh(inp @ w1[e] + b1[e]) @ w2[e] + b2[e])

Strategy (expert parallelism, `build_moe`): core e owns expert e's
weights (bf16). The gate runs data-parallel in exact fp32 (each core
gates its own 512 tokens; the tightest 2nd-vs-3rd logit margin in this
data is 6e-8, so top-2 selection must match the reference's fp32
bit-for-bit — the PE fp32 matmul does). Top-2 (idx0, idx1, w0, w1) per
token is AllGathered (8KB/core), from which every core derives its own
expert's mask + combine weight for all N tokens. Tokens are compacted
per half (2048 tokens -> <=640 slots) via matmul prefix-sum + ONE
multi-column indirect meta scatter, then a fused dma_gather(transpose)
pulls the selected x rows from DRAM directly into the transposed
[128, DC, 640] bf16 layout layer 1 wants. The 2-layer gelu FFN runs in
bf16 (PE full rate), layer-2 output is gate-scaled and dma_scatter_add
-ed into a zero-filled bf16 [2048, D] per-half partial; a
ReduceScatter(add) per half (the second overlapping the other half's
FFN) leaves each core with 2x256 output rows, reassembled on host.

`build_dense` (unused fallback) is the routing-free data-parallel
variant: every core computes all 8 experts for its 512 tokens.
"""
import numpy as np

import concourse.bacc as bacc
import concourse.bass as bass
import concourse.mybir as mybir
import concourse.tile as tile
from concourse.bass_utils import run_bass_kernel_spmd
from concourse.masks import make_identity

N, D, H, E, TOPK = 4096, 512, 2048, 8, 2
M = 8              # cores
TN = N // M        # tokens per core
P = 128
DC = D // P        # 4 contraction chunks over D
HC = H // P        # 16 chunks over H
TC = TN // P       # 4 token tiles per core
NT = N // P        # 32 token tiles total

NH = N // 2        # tokens per half (2048)
HT = NT // 2       # 16 token tiles per half
CAPH = 640         # compact slots per half (max observed load 559)
SCH = CAPH // P    # 5 compact tiles per half
CCS = [(0, 384), (384, 640)]   # layer-1 moving-dim chunks (PSUM bank <=512 fp32)
BIG = 8192.0       # OOB sentinel for unselected tokens

FP32 = mybir.dt.float32
BF16 = mybir.dt.bfloat16
I16 = mybir.dt.int16
I32 = mybir.dt.int32

AFT = mybir.ActivationFunctionType


DEBUG = False


def build_moe():
    nc = bacc.Bacc(None, target_bir_lowering=False)

    xT_own = nc.dram_tensor("xT_own", [D, N], FP32, kind="ExternalInput")
    x_bf = nc.dram_tensor("x_bf", [N, D], BF16, kind="ExternalInput")
    gate_w = nc.dram_tensor("gate_w", [D, E], FP32, kind="ExternalInput")
    gate_b = nc.dram_tensor("gate_b", [1, E], FP32, kind="ExternalInput")
    w1h_in = nc.dram_tensor("w1h_in", [P, HC, DC, P], BF16, kind="ExternalInput")
    b1t_in = nc.dram_tensor("b1t_in", [P, HC], FP32, kind="ExternalInput")
    w2e = nc.dram_tensor("w2e", [H, D], BF16, kind="ExternalInput")
    b2r_in = nc.dram_tensor("b2r_in", [1, D], BF16, kind="ExternalInput")
    ones_in = nc.dram_tensor("ones_in", [1, P], BF16, kind="ExternalInput")
    triu_in = nc.dram_tensor("triu_in", [P, P], FP32, kind="ExternalInput")
    tokid_in = nc.dram_tensor("tokid_in", [P, NT], FP32, kind="ExternalInput")
    dumpc_in = nc.dram_tensor("dumpc_in", [P, NT], FP32, kind="ExternalInput")
    dump16_in = nc.dram_tensor("dump16_in", [16, CAPH // 16], FP32,
                               kind="ExternalInput")
    b16_in = nc.dram_tensor("b16_in", [16, P], FP32, kind="ExternalInput")
    eid_in = nc.dram_tensor("eid_in", [P, 1], FP32, kind="ExternalInput")

    # compact meta: rows [0, CAPH) = slots, rows [CAPH, CAPH+NH) = dump for
    # unselected tokens. Lane 0 = tokid, lane 1 = gate weight (256B rows for
    # dma_scatter_add's elem-size floor).
    cmetas = [nc.dram_tensor(f"cmeta{h}", [CAPH + NH, 64], FP32)
              for h in range(2)]
    offds = [nc.dram_tensor(f"offd{h}", [NH], FP32) for h in range(2)]
    # rows [NH, NH+P) are a dump area for pad-slot writes: concurrent CCE adds
    # to one row are read-modify-write and can drop a racing real add, so pads
    # must never share a row with real tokens.
    partials = [nc.dram_tensor(f"partial{h}", [NH + P, D], BF16)
                for h in range(2)]
    rss = [nc.dram_tensor(f"rs{h}", [NH // M, D], BF16) for h in range(2)]
    outs = [nc.dram_tensor(f"o{h}", [NH // M, D], BF16, kind="ExternalOutput")
            for h in range(2)]
    if DEBUG:
        d_msb = nc.dram_tensor("d_msb", [P, 2, SCH, 2], FP32, kind="ExternalOutput")
        d_idx = nc.dram_tensor("d_idx", [P, 2, CAPH // 16], I16,
                               kind="ExternalOutput")
        d_xtg = nc.dram_tensor("d_xtg", [P, 2, DC, CAPH], BF16,
                               kind="ExternalOutput")
        d_y = nc.dram_tensor("d_y", [P, 2, SCH, D], BF16, kind="ExternalOutput")
        d_part = nc.dram_tensor("d_part", [P, 2, D], BF16, kind="ExternalOutput")

    with tile.TileContext(nc) as tc:
        with (
            tc.tile_pool(name="const", bufs=1) as const,
            tc.tile_pool(name="xsp", bufs=DC) as xsp,
            tc.tile_pool(name="gatep", bufs=2) as gatep,
            tc.tile_pool(name="routep", bufs=1) as routep,
            tc.tile_pool(name="w1p", bufs=HC) as w1p,
            tc.tile_pool(name="w2p", bufs=HC) as w2p,
            tc.tile_pool(name="xtgp", bufs=2) as xtgp,
            tc.tile_pool(name="hp", bufs=2 * HC) as hp,
            tc.tile_pool(name="yp", bufs=2) as yp,
            tc.tile_pool(name="psG", bufs=2, space="PSUM") as psG,
            tc.tile_pool(name="ps1", bufs=3, space="PSUM") as ps1,
            tc.tile_pool(name="ps2", bufs=3, space="PSUM") as ps2,
        ):
            # ---- gate input first: it heads the critical path ----
            gws = []
            for dc in range(DC):
                g = const.tile([P, E], FP32, tag=f"gw{dc}")
                nc.sync.dma_start(g[:], gate_w[dc * P:(dc + 1) * P, :])
                gws.append(g)
            gb = const.tile([1, E], FP32)
            nc.sync.dma_start(gb[:], gate_b[:])

            # ---- constants ----
            ones_row = const.tile([1, TN], FP32)
            nc.vector.memset(ones_row[:], 1.0)
            ones_col = const.tile([P, 1], FP32)
            nc.vector.memset(ones_col[:], 1.0)
            ones_s = const.tile([1, P], FP32)
            nc.vector.memset(ones_s[:], 1.0)
            ones_r = const.tile([1, P], BF16)
            nc.sync.dma_start(ones_r[:], ones_in[:])
            ident = const.tile([P, P], FP32)
            make_identity(nc, ident[:])
            triu = const.tile([P, P], FP32)
            nc.sync.dma_start(triu[:], triu_in[:])
            tokid = const.tile([P, NT], FP32)
            nc.sync.dma_start(tokid[:], tokid_in[:])
            dumpc = const.tile([P, NT], FP32)
            nc.sync.dma_start(dumpc[:], dumpc_in[:])
            dump16 = const.tile([16, CAPH // 16], FP32)
            nc.sync.dma_start(dump16[:], dump16_in[:])
            b16 = const.tile([16, P], FP32)
            nc.sync.dma_start(b16[:], b16_in[:])
            eidf = const.tile([P, 1], FP32)
            nc.sync.dma_start(eidf[:], eid_in[:])
            eidu = const.tile([P, 1], mybir.dt.uint32)
            nc.vector.tensor_copy(eidu[:], eidf[:])
            b1t = const.tile([P, HC], FP32)
            nc.sync.dma_start(b1t[:], b1t_in[:])
            b2r = const.tile([1, D], BF16)
            nc.sync.dma_start(b2r[:], b2r_in[:])

            # zero-init meta slot rows + output partials (off critical path)
            zmeta = const.tile([P, SCH, 64], FP32)
            nc.vector.memset(zmeta[:], 0.0)
            for h in range(2):
                nc.sync.dma_start(
                    cmetas[h][0:CAPH].rearrange("(s p) c -> p s c", p=P),
                    zmeta[:])
            ztb = const.tile([P, D], BF16)
            nc.vector.memset(ztb[:], 0.0)
            for h in range(2):
                for j in range(NH // P):
                    nc.sync.dma_start(partials[h][j * P:(j + 1) * P, :], ztb[:])

            # resident expert weights (bf16)
            w2t = []
            for hh in range(HC):
                w = w2p.tile([P, D], BF16, tag="w2t")
                nc.sync.dma_start(w[:], w2e[hh * P:(hh + 1) * P, :])
                w2t.append(w)
            w1t = []
            for hh in range(HC):
                w = w1p.tile([P, DC, P], BF16, tag="w1t")
                nc.sync.dma_start(w[:], w1h_in[:, hh])
                w1t.append(w)

            # ---- replicated gate: all N tokens, exact fp32, 512-tok chunks ----
            m_pack = routep.tile([P, NT], FP32, tag="m_pack")
            wt_pack = routep.tile([P, NT], FP32, tag="wt_pack")
            for ch in range(N // TN):
                xts = []
                for dc in range(DC):
                    t_ = xsp.tile([P, TN], FP32, tag="xts")
                    nc.sync.dma_start(
                        t_[:],
                        xT_own[dc * P:(dc + 1) * P, ch * TN:(ch + 1) * TN])
                    xts.append(t_)
                psT = psG.tile([E, TN], FP32, tag="psG")
                for dc in range(DC):
                    nc.tensor.matmul(psT[:], gws[dc][:], xts[dc][:],
                                     start=(dc == 0), stop=False)
                nc.tensor.matmul(psT[:], gb[:], ones_row[:],
                                 start=False, stop=True)
                lgT = gatep.tile([E, TN], FP32, tag="lgT")
                nc.vector.tensor_copy(lgT[:], psT[:])

                mxp = gatep.tile([P, TC, 8], FP32, tag="mxp")
                ixp = gatep.tile([P, TC, 8], mybir.dt.uint32, tag="ixp")
                for k in range(TC):
                    plg = psG.tile([P, E], FP32, tag="psG")
                    nc.tensor.transpose(plg[:], lgT[:, k * P:(k + 1) * P],
                                        ident[:E, :E])
                    lg = gatep.tile([P, E], FP32, tag="lg")
                    nc.vector.tensor_copy(lg[:], plg[:])
                    nc.vector.max_with_indices(mxp[:, k, :], ixp[:, k, :], lg[:])

                csl = slice(ch * TC, (ch + 1) * TC)
                dlt = gatep.tile([P, TC], FP32, tag="dlt")
                nc.vector.tensor_sub(dlt[:], mxp[:, :, 1], mxp[:, :, 0])
                e1 = gatep.tile([P, TC], FP32, tag="e1")
                nc.scalar.activation(e1[:], dlt[:], AFT.Exp)
                den = gatep.tile([P, TC], FP32, tag="den")
                nc.vector.tensor_scalar_add(den[:], e1[:], 1.0)
                w0 = gatep.tile([P, TC], FP32, tag="w0")
                nc.vector.reciprocal(w0[:], den[:])
                w1_ = gatep.tile([P, TC], FP32, tag="w1_")
                nc.vector.tensor_mul(w1_[:], e1[:], w0[:])
                h0 = gatep.tile([P, TC], FP32, tag="h0")
                nc.vector.tensor_tensor(
                    out=h0[:], in0=ixp[:, :, 0],
                    in1=eidu[:].to_broadcast([P, TC]),
                    op=mybir.AluOpType.is_equal)
                h1 = gatep.tile([P, TC], FP32, tag="h1")
                nc.vector.tensor_tensor(
                    out=h1[:], in0=ixp[:, :, 1],
                    in1=eidu[:].to_broadcast([P, TC]),
                    op=mybir.AluOpType.is_equal)
                nc.vector.tensor_add(m_pack[:, csl], h0[:], h1[:])
                nc.vector.tensor_mul(h0[:], h0[:], w0[:])
                nc.vector.tensor_mul(h1[:], h1[:], w1_[:])
                nc.vector.tensor_add(wt_pack[:, csl], h0[:], h1[:])

            # ---- routing per half ----
            # prefix-sum -> per-token slot (unselected -> dump region) ->
            # 16-wrap idx via DRAM bounce + PE replicate -> ONE meta
            # dma_scatter_add -> slot->tokid idx -> fused gather+transpose.
            xtgs, msbs, idxs, idxs_s = [], [], [], []
            for half in range(2):
                hsl = slice(HT * half, HT * (half + 1))
                p_tot = psG.tile([HT, 1], FP32, tag="psG")
                nc.tensor.matmul(p_tot[:], m_pack[:, hsl], ones_col[:],
                                 start=True, stop=True)
                totT = routep.tile([HT, 1], FP32, tag=f"totT{half}")
                nc.vector.tensor_copy(totT[:], p_tot[:])
                p_srow = psG.tile([1, HT], FP32, tag="psG")
                nc.tensor.matmul(p_srow[:], totT[:], triu[0:HT, 0:HT],
                                 start=True, stop=True)
                s_row = routep.tile([1, HT], FP32, tag=f"srow{half}")
                nc.vector.tensor_copy(s_row[:], p_srow[:])
                p_pl = psG.tile([P, HT], FP32, tag="psG")
                nc.tensor.matmul(p_pl[:], triu[:], m_pack[:, hsl],
                                 start=True, stop=False)
                nc.tensor.matmul(p_pl[:], ones_s[:], s_row[:], start=False, stop=True)

                # off = m*slot + (1-m)*(CAPH + tokid)  (per token, fp32)
                off_f = routep.tile([P, HT], FP32, tag=f"offf{half}")
                nc.vector.tensor_sub(off_f[:], p_pl[:], dumpc[:, hsl])
                nc.vector.tensor_mul(off_f[:], off_f[:], m_pack[:, hsl])
                nc.vector.tensor_add(off_f[:], off_f[:], dumpc[:, hsl])
                # DRAM bounce into token order, reload 16-wrapped
                nc.sync.dma_start(
                    offds[half].rearrange("(t p) -> p t", p=P), off_f[:])
                offw = routep.tile([16, P], FP32, tag=f"offw{half}")
                nc.sync.dma_start(
                    offw[:], offds[half].rearrange("(m q) -> q m", q=16))
                ps_sx = psG.tile([P, P], FP32, tag="psG")
                nc.tensor.matmul(ps_sx[:], b16[:], offw[:], start=True, stop=True)
                idx_sx = routep.tile([P, P], I16, tag=f"idxsx{half}")
                nc.vector.tensor_copy(idx_sx[:], ps_sx[:])

                # meta payload: lane0 = tokid, lane1 = gate weight
                vals64 = routep.tile([P, HT, 64], FP32, tag=f"vals{half}")
                nc.vector.memset(vals64[:], 0.0)
                nc.vector.tensor_copy(vals64[:, :, 0], tokid[:, hsl])
                nc.vector.tensor_copy(vals64[:, :, 1], wt_pack[:, hsl])
                nc.gpsimd.dma_scatter_add(
                    cmetas[half][:], vals64[:], idx_sx[:], NH, NH, 64)

                # meta back: weights in 128-wrap, tokids in 16-wrap
                msb = routep.tile([P, SCH, 64], FP32, tag=f"msb{half}")
                nc.sync.dma_start(
                    msb[:], cmetas[half][0:CAPH].rearrange("(s p) c -> p s c", p=P))
                msbs.append(msb)
                m16 = routep.tile([16, CAPH // 16, 64], FP32, tag=f"m16_{half}")
                nc.sync.dma_start(
                    m16[:], cmetas[half][0:CAPH].rearrange("(s p) c -> p s c", p=16))
                mt = routep.tile([16, CAPH // 16], FP32, tag=f"mt{half}")
                nc.vector.tensor_copy(mt[:], m16[:, :, 0])
                ps_g = psG.tile([P, CAPH // 16], FP32, tag="psG")
                nc.tensor.matmul(ps_g[:], b16[:], mt[:], start=True, stop=True)
                idx_g = routep.tile([P, CAPH // 16], I16, tag=f"idxg{half}")
                nc.vector.tensor_copy(idx_g[:], ps_g[:])
                idxs.append(idx_g)
                # scatter idx: pads (wt==0) diverted to the dump rows
                pad16 = routep.tile([16, CAPH // 16], FP32, tag=f"pad16_{half}")
                nc.vector.tensor_scalar(pad16[:], m16[:, :, 1], 0.0, None,
                                        op0=mybir.AluOpType.is_equal)
                nc.vector.tensor_mul(pad16[:], pad16[:], dump16[:])
                mts = routep.tile([16, CAPH // 16], FP32, tag=f"mts{half}")
                nc.vector.tensor_add(mts[:], mt[:], pad16[:])
                ps_s = psG.tile([P, CAPH // 16], FP32, tag="psG")
                nc.tensor.matmul(ps_s[:], b16[:], mts[:], start=True, stop=True)
                idx_s = routep.tile([P, CAPH // 16], I16, tag=f"idxs{half}")
                nc.vector.tensor_copy(idx_s[:], ps_s[:])
                idxs_s.append(idx_s)

                xtg = xtgp.tile([P, DC, CAPH], BF16, tag="xtg")
                nc.gpsimd.dma_gather(
                    xtg[:], x_bf[NH * half:NH * (half + 1), :], idx_g[:],
                    CAPH, CAPH, D, transpose=True)
                xtgs.append(xtg)
                if DEBUG:
                    nc.sync.dma_start(d_msb[:, half], msb[:, :, 0:2])
                    nc.sync.dma_start(d_idx[:, half], idx_g[:])
                    nc.sync.dma_start(d_xtg[:, half], xtg[:])

            # ---- FFN per half (bf16), scatter-add, ReduceScatter ----
            for half in range(2):
                xtg, msb, idx16 = xtgs[half], msbs[half], idxs_s[half]
                hts = []
                for hh in range(HC):
                    ht = hp.tile([P, CAPH], BF16, tag="ht")
                    pcs = [ps1.tile([P, c1 - c0], FP32, tag="ps1", name=f"pcs{ci}")
                           for ci, (c0, c1) in enumerate(CCS)]
                    for dc in range(DC):
                        for ci, (c0, c1) in enumerate(CCS):
                            nc.tensor.matmul(
                                pcs[ci][:], w1t[hh][:, dc, :], xtg[:, dc, c0:c1],
                                start=(dc == 0), stop=(dc == DC - 1))
                    for ci, (c0, c1) in enumerate(CCS):
                        nc.scalar.activation(ht[:, c0:c1], pcs[ci][:],
                                             AFT.Gelu_apprx_tanh,
                                             bias=b1t[:, hh:hh + 1])
                    hts.append(ht)

                y = yp.tile([P, SCH, D], BF16, tag="y")
                for s in range(SCH):
                    p2 = ps2.tile([P, D], FP32, tag="ps2")
                    for hh in range(HC):
                        nc.tensor.matmul(p2[:], hts[hh][:, s * P:(s + 1) * P],
                                         w2t[hh][:], start=(hh == 0), stop=False)
                    nc.tensor.matmul(p2[:], ones_r[:], b2r[:],
                                     start=False, stop=True)
                    nc.scalar.activation(y[:, s, :], p2[:], AFT.Copy,
                                         scale=msb[:, s, 1:2])

                if DEBUG:
                    nc.sync.dma_start(d_y[:, half], y[:])
                nc.gpsimd.dma_scatter_add(
                    partials[half][:], y[:], idx16[:], CAPH, CAPH, D)
                if DEBUG:
                    pb = yp.tile([P, D], BF16, tag="pb")
                    nc.sync.dma_start(pb[:], partials[half][0:P, :])
                    nc.sync.dma_start(d_part[:, half], pb[:])
                nc.gpsimd.collective_compute(
                    "ReduceScatter", mybir.AluOpType.add,
                    replica_groups=[list(range(M))],
                    ins=[partials[half][0:NH].opt()], outs=[rss[half][:].opt()])
                for j in range(NH // M // P):
                    ob = yp.tile([P, D], BF16, tag="ob")
                    nc.sync.dma_start(ob[:], rss[half][j * P:(j + 1) * P, :])
                    nc.sync.dma_start(outs[half][j * P:(j + 1) * P, :], ob[:])

    nc.compile()
    return nc


def make_moe_in_maps(inp, gate_w, gate_b, w1, b1, w2, b2):
    import ml_dtypes
    bf16 = ml_dtypes.bfloat16
    inp = np.ascontiguousarray(np.asarray(inp, dtype=np.float32))
    gate_w = np.ascontiguousarray(np.asarray(gate_w, dtype=np.float32))
    gate_b = np.ascontiguousarray(np.asarray(gate_b, dtype=np.float32)).reshape(1, E)
    w1 = np.asarray(w1, dtype=np.float32)
    b1 = np.asarray(b1, dtype=np.float32)
    w2 = np.asarray(w2, dtype=np.float32)
    b2 = np.asarray(b2, dtype=np.float32)

    x_bf = np.ascontiguousarray(inp.astype(bf16))
    xT = np.ascontiguousarray(inp.T)
    triu = np.triu(np.ones((P, P), np.float32), k=1)
    # token id within its half: tile t holds tokens (t%16)*128+p of half t//16
    tokid = ((np.arange(NT)[None, :] % HT) * P
             + np.arange(P)[:, None]).astype(np.float32)
    dumpc = tokid + CAPH
    # pad-slot scatter target: NH + slot%P, distinct rows past the RS window
    slot16 = (np.arange(CAPH // 16)[None, :] * 16 + np.arange(16)[:, None])
    dump16 = (NH + slot16 % P).astype(np.float32)
    # replication matrix: b16[k, i] = 1 iff i % 16 == k (16->128 partition bcast)
    b16 = (np.arange(P)[None, :] % 16 == np.arange(16)[:, None]).astype(np.float32)
    ones = np.ones((1, P), np.float32).astype(bf16)

    in_maps = []
    for c in range(M):
        w1h = np.ascontiguousarray(
            w1[c].reshape(DC, P, HC, P).transpose(1, 2, 0, 3).astype(bf16))
        in_maps.append({
            "xT_own": xT,
            "x_bf": x_bf,
            "gate_w": gate_w, "gate_b": gate_b,
            "w1h_in": w1h,
            "b1t_in": np.ascontiguousarray(b1[c].reshape(HC, P).T),
            "w2e": np.ascontiguousarray(w2[c].astype(bf16)),
            "b2r_in": np.ascontiguousarray(b2[c].reshape(1, D).astype(bf16)),
            "ones_in": ones,
            "triu_in": triu,
            "tokid_in": tokid,
            "dumpc_in": dumpc,
            "dump16_in": dump16,
            "b16_in": b16,
            "eid_in": np.full((P, 1), c, np.float32),
        })
    return in_maps


# ---------------------------------------------------------------------------
# Fallback: dense data-parallel variant (every core runs all 8 experts on its
# 512 tokens). Unused unless KERNEL_KIND is changed.
# ---------------------------------------------------------------------------

def _gate_combine(nc, tc_ctx, pools, xts, gws, gb, ones_s, iota_u, n_tok_chunks):
    gatep, cmbp, psg = pools
    U32 = mybir.dt.uint32
    TNW = n_tok_chunks * P
    ones_row = gatep.tile([1, TNW], FP32, tag="ones_row")
    nc.vector.memset(ones_row[:], 1.0)
    ident = gatep.tile([P, P], FP32, tag="ident_g")
    make_identity(nc, ident[:])
    psT = psg.tile([E, TNW], FP32, tag="psg")
    for dc in range(len(xts)):
        nc.tensor.matmul(psT[:], gws[dc][:], xts[dc][:, 0:TNW],
                         start=(dc == 0), stop=False)
    nc.tensor.matmul(psT[:], gb[:], ones_row[:], start=False, stop=True)
    lgT = gatep.tile([E, TNW], FP32, tag="lgT")
    nc.scalar.activation(lgT[:], psT[:], AFT.Copy)

    cmb = []
    cmbT = []
    for t in range(n_tok_chunks):
        pg = psg.tile([P, E], FP32, tag="psg")
        nc.tensor.transpose(pg[:], lgT[:, t * P:(t + 1) * P], ident[:E, :E])

        lg = gatep.tile([P, E], FP32, tag="lg")
        nc.vector.tensor_copy(lg[:], pg[:])
        mx = gatep.tile([P, 8], FP32, tag="mx")
        ix = gatep.tile([P, 8], U32, tag="ix")
        nc.vector.max_with_indices(mx[:], ix[:], lg[:])

        dlt = gatep.tile([P, 1], FP32, tag="dlt")
        nc.vector.tensor_sub(dlt[:], mx[:, 1:2], mx[:, 0:1])
        e1 = gatep.tile([P, 1], FP32, tag="e1")
        nc.scalar.activation(e1[:], dlt[:], AFT.Exp)
        den = gatep.tile([P, 1], FP32, tag="den")
        nc.vector.tensor_scalar_add(den[:], e1[:], 1.0)
        w0 = gatep.tile([P, 1], FP32, tag="w0")
        nc.vector.reciprocal(w0[:], den[:])
        w1_ = gatep.tile([P, 1], FP32, tag="w1_")
        nc.vector.tensor_mul(w1_[:], e1[:], w0[:])

        oh0 = gatep.tile([P, E], FP32, tag="oh0")
        nc.vector.tensor_tensor(out=oh0[:], in0=ix[:, 0:1].to_broadcast([P, E]),
                                in1=iota_u[:], op=mybir.AluOpType.is_equal)
        oh1 = gatep.tile([P, E], FP32, tag="oh1")
        nc.vector.tensor_tensor(out=oh1[:], in0=ix[:, 1:2].to_broadcast([P, E]),
                                in1=iota_u[:], op=mybir.AluOpType.is_equal)
        nc.vector.tensor_scalar_mul(oh0[:], oh0[:], w0[:, 0:1])
        nc.vector.tensor_scalar_mul(oh1[:], oh1[:], w1_[:, 0:1])
        c = cmbp.tile([P, E], FP32, tag="cmb")
        nc.vector.tensor_add(c[:], oh0[:], oh1[:])
        cmb.append(c)
        pct = psg.tile([E, P], FP32, tag="psg")
        nc.tensor.transpose(pct[:], c[:], ident[:])
        ct = cmbp.tile([E, P], BF16, tag="cmbT")
        nc.vector.tensor_copy(ct[:], pct[:])
        cmbT.append(ct)
    return cmb, cmbT


def build_dense():
    nc = bacc.Bacc(None, target_bir_lowering=False)
    U32 = mybir.dt.uint32

    xT_r = nc.dram_tensor("xT_r", [D, TN], BF16, kind="ExternalInput")
    xT_s = nc.dram_tensor("xT_s", [D, TN], FP32, kind="ExternalInput")
    gate_w = nc.dram_tensor("gate_w", [D, E], FP32, kind="ExternalInput")
    gate_b = nc.dram_tensor("gate_b", [1, E], FP32, kind="ExternalInput")
    w1 = nc.dram_tensor("w1", [E, D, H], BF16, kind="ExternalInput")
    b1p = nc.dram_tensor("b1p", [E, P, HC], FP32, kind="ExternalInput")
    w2 = nc.dram_tensor("w2", [E, H, D], BF16, kind="ExternalInput")
    b2 = nc.dram_tensor("b2", [E, 1, D], BF16, kind="ExternalInput")
    ones_in = nc.dram_tensor("ones_in", [1, P], BF16, kind="ExternalInput")
    out = nc.dram_tensor("out", [TN, D], FP32, kind="ExternalOutput")

    with tile.TileContext(nc) as tc:
        with (
            tc.tile_pool(name="xpool", bufs=DC) as xpool,
            tc.tile_pool(name="const", bufs=1) as const,
            tc.tile_pool(name="gatep", bufs=2) as gatep,
            tc.tile_pool(name="cmbp", bufs=TC) as cmbp,
            tc.tile_pool(name="w1p", bufs=6) as w1p,
            tc.tile_pool(name="w2p", bufs=2 * HC) as w2p,
            tc.tile_pool(name="hp", bufs=2 * HC) as hp,
            tc.tile_pool(name="accp", bufs=TC) as accp,
            tc.tile_pool(name="tmpp", bufs=3) as tmpp,
            tc.tile_pool(name="bp", bufs=4) as bp,
            tc.tile_pool(name="psg", bufs=1, space="PSUM") as psg,
            tc.tile_pool(name="ps1", bufs=3, space="PSUM") as ps1,
            tc.tile_pool(name="ps2", bufs=3, space="PSUM") as ps2,
        ):
            xtr, xts = [], []
            for dc in range(DC):
                tr = xpool.tile([P, TN], BF16, tag="xtr")
                nc.sync.dma_start(tr[:], xT_r[dc * P:(dc + 1) * P, :])
                xtr.append(tr)
                ts = xpool.tile([P, TN], FP32, tag="xts")
                nc.sync.dma_start(ts[:], xT_s[dc * P:(dc + 1) * P, :])
                xts.append(ts)

            ones_s = const.tile([1, P], FP32)
            nc.vector.memset(ones_s[:], 1.0)
            ones_r = const.tile([1, P], BF16)
            nc.sync.dma_start(ones_r[:], ones_in[:])
            iota_u = const.tile([P, E], U32)
            nc.gpsimd.iota(iota_u[:], pattern=[[1, E]], base=0, channel_multiplier=0)

            gws = []
            for dc in range(DC):
                g = const.tile([P, E], FP32, tag=f"gw{dc}")
                nc.sync.dma_start(g[:], gate_w[dc * P:(dc + 1) * P, :])
                gws.append(g)
            gb = const.tile([1, E], FP32)
            nc.sync.dma_start(gb[:], gate_b[:])

            cmb, cmbT = _gate_combine(nc, tc, (gatep, cmbp, psg), xts, gws, gb,
                                      ones_s, iota_u, TC)
            b2all = bp.tile([E, D], BF16, tag="b2all")
            nc.sync.dma_start(b2all[:], b2[:, 0, :])

            acc = [None] * TC
            for e in range(E):
                w2t = []
                for h in range(HC):
                    w = w2p.tile([P, D], BF16, tag="w2t")
                    nc.sync.dma_start(w[:], w2[e, h * P:(h + 1) * P, :])
                    w2t.append(w)
                b1te = bp.tile([P, HC], FP32, tag="b1t")
                nc.sync.dma_start(b1te[:], b1p[e])

                hts = []
                w1e = w1[e].rearrange("(dc p) h -> p dc h", p=P)
                for h in range(HC):
                    w1te = w1p.tile([P, DC, P], BF16, tag="w1t")
                    nc.sync.dma_start(w1te[:], w1e[:, :, h * P:(h + 1) * P])
                    p1 = ps1.tile([P, TN], FP32)
                    for dc in range(DC):
                        nc.tensor.matmul(p1[:], w1te[:, dc, :], xtr[dc][:],
                                         start=(dc == 0), stop=(dc == DC - 1))
                    ht = hp.tile([P, TN], BF16, tag="ht")
                    nc.scalar.activation(ht[:], p1[:], AFT.Gelu_apprx_tanh,
                                         bias=b1te[:, h:h + 1])
                    hts.append(ht)

                for t in range(TC):
                    p2 = ps2.tile([P, D], FP32)
                    for h in range(HC):
                        nc.tensor.matmul(p2[:], hts[h][:, t * P:(t + 1) * P], w2t[h][:],
                                         start=(h == 0), stop=(h == HC - 1))
                    if e == 0:
                        a = accp.tile([P, D], FP32, tag="acc")
                        nc.vector.tensor_scalar_mul(a[:], p2[:], cmb[t][:, e:e + 1])
                        acc[t] = a
                    else:
                        tmp = tmpp.tile([P, D], FP32, tag="tmp")
                        nc.scalar.activation(tmp[:], p2[:], AFT.Copy,
                                             scale=cmb[t][:, e:e + 1])
                        nc.vector.tensor_add(acc[t][:], acc[t][:], tmp[:])

            for t in range(TC):
                pB = ps2.tile([P, D], FP32, tag="p2")
                nc.tensor.matmul(pB[:], cmbT[t][:], b2all[:], start=True, stop=True)
                nc.vector.tensor_add(acc[t][:], acc[t][:], pB[:])
                nc.sync.dma_start(out[t * P:(t + 1) * P, :], acc[t][:])

    nc.compile()
    return nc


def make_in_maps(inp, gate_w, gate_b, w1, b1, w2, b2):
    import ml_dtypes
    bf16 = ml_dtypes.bfloat16
    inp = np.ascontiguousarray(np.asarray(inp, dtype=np.float32))
    gate_w = np.ascontiguousarray(np.asarray(gate_w, dtype=np.float32))
    gate_b = np.ascontiguousarray(np.asarray(gate_b, dtype=np.float32)).reshape(1, E)
    w1 = np.ascontiguousarray(np.asarray(w1, dtype=np.float32).astype(bf16))
    b1 = np.asarray(b1, dtype=np.float32)
    w2 = np.ascontiguousarray(np.asarray(w2, dtype=np.float32).astype(bf16))
    b2 = np.ascontiguousarray(
        np.asarray(b2, dtype=np.float32).astype(bf16)).reshape(E, 1, D)
    b1p = np.ascontiguousarray(b1.reshape(E, HC, P).transpose(0, 2, 1))

    in_maps = []
    for c in range(M):
        xT = np.ascontiguousarray(inp[c * TN:(c + 1) * TN, :].T)
        in_maps.append({
            "xT_r": np.ascontiguousarray(xT.astype(bf16)), "xT_s": xT,
            "gate_w": gate_w, "gate_b": gate_b,
            "w1": w1, "b1p": b1p, "w2": w2, "b2": b2,
            "ones_in": np.ones((1, P), np.float32).astype(bf16),
        })
    return in_maps


_NC_CACHE = {}

# "dense" (286us) still beats the expert-parallel "moe" path (325-358us):
# the moe FFN itself is ~4x cheaper, but collective setup (~15-30us each),
# serial gpsimd scatter/gather desc-gen, and routing latency dominate.
KERNEL_KIND = "moe"


def _get_nc():
    if KERNEL_KIND not in _NC_CACHE:
        _NC_CACHE[KERNEL_KIND] = (
            build_moe() if KERNEL_KIND == "moe" else build_dense())
    return _NC_CACHE[KERNEL_KIND]


def run(inputs, trace=False, **spmd_kwargs):
    nc = _get_nc()
    mk = make_moe_in_maps if KERNEL_KIND == "moe" else make_in_maps
    in_maps = mk(
        inputs["inp"], inputs["gate_w"], inputs["gate_b"],
        inputs["w1"], inputs["b1"], inputs["w2"], inputs["b2"])
    res = run_bass_kernel_spmd(nc, in_maps, list(range(M)), trace=trace,
                               **spmd_kwargs)
    if KERNEL_KIND == "moe":
        h0 = np.concatenate(
            [np.asarray(res.results[c]["o0"], np.float32) for c in range(M)], axis=0)
        h1 = np.concatenate(
            [np.asarray(res.results[c]["o1"], np.float32) for c in range(M)], axis=0)
        out = np.concatenate([h0, h1], axis=0)
    else:
        out = np.concatenate([res.results[c]["out"] for c in range(M)], axis=0)
    return out, res


def kernel(inp, gate_w, gate_b, w1, b1, w2, b2, top_k):
    assert int(top_k) == TOPK
    out, _ = run({"inp": inp, "gate_w": gate_w, "gate_b": gate_b,
                  "w1": w1, "b1": b1, "w2": w2, "b2": b2})
    return out

